# revision 18
# baseline (speedup 1.0000x reference)
"""Trainium2 Bass kernel for a BERT layer with relative-position attention bias.

Contract: kernel(**inputs) takes the FULL inputs (as produced by the problem's
setup_inputs) and returns the FULL output [8, 512, 768] float32.

Strategy: data-parallel over batch (B=8 -> one batch element per NeuronCore),
weights replicated, no collectives. Per-core dataflow:

  - activations kept feature-major ([H, S]) for Q/K and the FFN intermediate,
    token-major ([S, H]) for V / attn-out / layernorms.
  - scores computed k-major (scoresT[k, q]) so softmax normalization is a
    per-head partition-broadcast multiply and the context matmul consumes
    exp(scores) directly (no probs transpose).
  - relative-position bias via the Toeplitz/shift trick: per (head, q-block)
    A = Q_blk^T @ RT window [128, 640] -> DRAM (fp8) -> shifted strided DMA
    read back as B[q, k] [128, 512] -> transposed-accumulated into the scores
    PSUM with identity matmuls.  All 12 heads' bias pipelines run during the
    Q/K projections; the 12 fp8 B tiles stay resident so attention never
    stalls on the DRAM roundtrip.
  - softmax denominator accumulated by the context matmul itself via an
    interleaved ones-column in V (65 columns per head).
  - softmax without max-subtraction (scores are O(1); same math).
  - matmuls in fp16 (fast weight loads, ~3e-4 matmul rel err); fp32
    accumulation in PSUM, fp32 layernorm/residual arithmetic.
  - Wo accumulated kb-outer in two 4-bank passes so the last head pair's
    normalize chain is hidden; FFN2 runs tb-outer so LN2 + output DMA
    pipeline with the remaining matmuls.
"""
import os
import sys

for _p in ("/opt/trn_rl_repo", os.path.expanduser("~/.axon_site/_ro/trn_rl_repo")):
    if os.path.isdir(_p) and _p not in sys.path:
        sys.path.insert(0, _p)

import numpy as np
import ml_dtypes

import concourse.bass as bass
import concourse.mybir as mybir
import concourse.tile as tile
from concourse import bacc
from concourse.bass_utils import run_bass_kernel_spmd

P = 128
S = 512
H = 768
NH = 12
HD = 64
FF = 3072
MAXPOS = 512
EPS = 1e-12
HB = H // P       # 6 feature blocks
TB = S // P       # 4 token blocks
FB = FF // P      # 24 ff blocks
NJ = 640          # rel window width per q-block
OFF = 127         # shift-read column offset
VW = NH * (HD + 1)  # V row width: 12 heads x (64 value cols + 1 ones col)

F32 = mybir.dt.float32
F16 = mybir.dt.float16
F8 = mybir.dt.float8e4

AFT = mybir.ActivationFunctionType
ALU = mybir.AluOpType


def build(trivial_ln1: bool, trivial_ln2: bool):
    nc = bacc.Bacc("TRN2", target_bir_lowering=False, debug=False)

    # ---------------- DRAM I/O ----------------
    d_xT = nc.dram_tensor("xT", [P, HB, S], F16, kind="ExternalInput")
    d_x = nc.dram_tensor("x_res", [S, H], F16, kind="ExternalInput")
    d_wq = nc.dram_tensor("wq", [P, HB, HB, P], F16, kind="ExternalInput")
    d_wk = nc.dram_tensor("wk", [P, HB, H], F16, kind="ExternalInput")
    d_wv = nc.dram_tensor("wv", [P, HB, H], F16, kind="ExternalInput")
    d_wo = nc.dram_tensor("wo", [P, HB, H], F16, kind="ExternalInput")
    d_w1 = nc.dram_tensor("w1", [P, HB, FF], F16, kind="ExternalInput")
    d_w2 = nc.dram_tensor("w2", [P, FB, H], F16, kind="ExternalInput")
    d_rt = nc.dram_tensor("rt", [P, 1024], F16, kind="ExternalInput")
    d_bc = nc.dram_tensor("bcombo", [P, 36], F32, kind="ExternalInput")
    d_rows = nc.dram_tensor("rows2", [1, H], F16, kind="ExternalInput")
    d_onesr = nc.dram_tensor("ones_row", [1, P], F16, kind="ExternalInput")
    d_idh = nc.dram_tensor("ident_f8", [P, P], F8, kind="ExternalInput")
    d_idf = nc.dram_tensor("ident_f32", [P, P], F32, kind="ExternalInput")
    if not trivial_ln1:
        d_l1s = nc.dram_tensor("ln1s_b", [P, H], F32, kind="ExternalInput")
        d_l1b = nc.dram_tensor("ln1b_b", [P, H], F32, kind="ExternalInput")
    if not trivial_ln2:
        d_l2s = nc.dram_tensor("ln2s_b", [P, H], F32, kind="ExternalInput")
        d_l2b = nc.dram_tensor("ln2b_b", [P, H], F32, kind="ExternalInput")
    d_out = nc.dram_tensor("out", [S, H], F32, kind="ExternalOutput")

    with tile.TileContext(nc) as tc:
        with (
            tc.tile_pool(name="const", bufs=1) as const,
            tc.tile_pool(name="persist", bufs=1) as persist,
            tc.tile_pool(name="wr", bufs=4) as wr_pool,
            tc.tile_pool(name="psm", bufs=5, space="PSUM") as psm,
            tc.tile_pool(name="psh", bufs=3, space="PSUM") as psh,
            tc.tile_pool(name="stat", bufs=4) as statp,
            tc.tile_pool(name="evict", bufs=2) as evp,
        ):
            # ---- PE warm-up: junk matmuls release the HAM clock throttle
            # while the first DMAs are still in flight.
            warm_w = const.tile([P, P], F16, name="warm_w")
            nc.gpsimd.memset(warm_w, 0.0)
            for wi in range(32):
                pw = psm.tile([P, P], F32, tag="m", name=f"warm_{wi}")
                nc.tensor.matmul(pw, warm_w, warm_w, start=True, stop=True)

            # ---- input + first-use weights.  Three DMA queues (sync=q1,
            # scalar=q10, gpsimd=q0); balance the Q-critical bytes across
            # sync+scalar with big descriptors (wq is host-tiled hb-major so
            # the first 2 output blocks arrive as their own chunk); gpsimd's
            # software-DGE queue carries the bulk low-urgency weights.
            xT_sb = persist.tile([P, HB, S], F16, name="xT_sb")
            wq_t = wr_pool.tile([P, HB, HB, P], F16, tag="s1", name="wq_t", bufs=1)
            wk_t = wr_pool.tile([P, HB, H], F16, tag="s2", name="wk_t", bufs=1)
            wv_t = wr_pool.tile([P, HB, H], F16, tag="s3", name="wv_t", bufs=1)
            nc.sync.dma_start(xT_sb[:, 0:3, :], d_xT.ap()[:, 0:3, :])
            nc.gpsimd.dma_start(xT_sb[:, 3:HB, :], d_xT.ap()[:, 3:HB, :])
            nc.scalar.dma_start(wq_t[:, 0:2], d_wq.ap()[:, 0:2])
            nc.sync.dma_start(wq_t[:, 2:HB], d_wq.ap()[:, 2:HB])
            # rt next on scalar (needed by the first bias matmuls ~13us)
            rt_sb = const.tile([P, 1024], F16, name="rt_sb")
            nc.scalar.dma_start(rt_sb, d_rt.ap())
            bc_sb = const.tile([P, 36], F32, name="bc_sb")
            nc.scalar.dma_start(bc_sb, d_bc.ap())
            idh_sb = const.tile([P, P], F8, name="idh_sb")
            nc.scalar.dma_start(idh_sb, d_idh.ap())
            nc.sync.dma_start(wk_t, d_wk.ap())
            nc.scalar.dma_start(wv_t, d_wv.ap())
            wq_sb = {(kb, hb): wq_t[:, hb, kb, :] for kb in range(HB) for hb in range(HB)}
            wk_sb = [wk_t[:, kb, :] for kb in range(HB)]
            wv_sb = [wv_t[:, kb, :] for kb in range(HB)]

            bq8_sb = bc_sb[:, 0:HB]
            bk_sb = bc_sb[:, HB : 2 * HB]
            b1_sb = bc_sb[:, 2 * HB : 2 * HB + FB]
            # low-urgency consts (transpose identity, FFN bias rows)
            rows_sb = const.tile([1, H], F16, name="rows_sb")
            onesr_sb = const.tile([1, P], F16, name="onesr_sb")
            idf_sb = const.tile([P, P], F32, name="idf_sb")
            b2_sb = rows_sb[:, 0:H]
            eps_sb = const.tile([P, 1], F32, name="eps_sb")
            nc.gpsimd.memset(eps_sb, EPS)
            if not trivial_ln1:
                l1s_sb = const.tile([P, H], F32, name="l1s_sb")
                nc.scalar.dma_start(l1s_sb, d_l1s.ap())
                l1b_sb = const.tile([P, H], F32, name="l1b_sb")
                nc.scalar.dma_start(l1b_sb, d_l1b.ap())
            if not trivial_ln2:
                l2s_sb = const.tile([P, H], F32, name="l2s_sb")
                nc.scalar.dma_start(l2s_sb, d_l2s.ap())
                l2b_sb = const.tile([P, H], F32, name="l2b_sb")
                nc.scalar.dma_start(l2b_sb, d_l2b.ap())

            # ---- persistent activations ----
            h1_sb = persist.tile([P, TB, H], F32, name="h1_sb")
            h1T_sb = persist.tile([P, HB, S], F16, name="h1T_sb")
            if not trivial_ln1:
                h1n_sb = persist.tile([P, TB, H], F32, name="h1n_sb")

            # ================= attention scope =================
            with (
                tc.tile_pool(name="attn", bufs=1) as ap_,
                tc.tile_pool(name="expool", bufs=3) as expool,
                tc.tile_pool(name="Apool", bufs=3) as Apool,
                tc.tile_pool(name="Bpool", bufs=12) as Bpool,
                tc.tile_pool(name="smallp", bufs=2) as smallp,
                tc.tile_pool(name="scr", bufs=12, space="DRAM") as scrp,
            ):
                x_sb = ap_.tile([P, TB, H], F16, name="x_sb")
                QT_sb = ap_.tile([P, HB, S], F16, name="QT_sb")
                KT_sb = ap_.tile([P, HB, S], F16, name="KT_sb")
                # V with an interleaved ones-column per head: head h occupies
                # columns [65h, 65h+64), column 65h+64 is ones so the context
                # matmul also produces the softmax denominator in row 64.
                V_sb = ap_.tile([P, TB, VW], F16, name="V_sb")
                nc.vector.memset(V_sb, 1.0)
                ctxT_sb = ap_.tile([P, HB, S], F16, name="ctxT_sb")

                def q_head(h):
                    return QT_sb[64 * (h % 2) : 64 * (h % 2) + 64, h // 2, :]

                def k_head(h):
                    return KT_sb[64 * (h % 2) : 64 * (h % 2) + 64, h // 2, :]

                B_tiles = {}
                A4_tiles = {}

                def emit_bias_block(hp, qb):
                    # one (head-pair, q-block) slice of the rel-bias pipeline:
                    # 3 PSUM tiles (pb1 x2 heads, shared pb2), 3 matmuls,
                    # 4 eviction casts split across vector + scalar.
                    heads = (2 * hp, 2 * hp + 1)
                    if qb == 0:
                        for h in heads:
                            A4_tiles[h] = Apool.tile(
                                [P, TB, NJ], F8, tag="A", name=f"A_{h}"
                            )
                    q0 = qb * P
                    j0 = 384 - q0
                    pbs = {}
                    for h in heads:
                        Qh = q_head(h)
                        b0 = 64 * (h % 2)
                        rth = rt_sb[b0 : b0 + HD, :]
                        pb1 = psh.tile(
                            [P, S], F32, tag="h", name=f"pb1_{h}_{qb}"
                        )
                        nc.tensor.matmul(
                            pb1, Qh[:, q0 : q0 + P], rth[:, j0 : j0 + S],
                            start=True, stop=True,
                        )
                        pbs[h] = pb1
                    for h in heads:
                        Qh = q_head(h)
                        b0 = 64 * (h % 2)
                        rth = rt_sb[b0 : b0 + HD, :]
                        pb2 = psh.tile(
                            [P, S], F32, tag="h", name=f"pb2_{h}_{qb}"
                        )
                        nc.tensor.matmul(
                            pb2[:, 0:P], Qh[:, q0 : q0 + P],
                            rth[:, j0 + S : j0 + NJ],
                            start=True, stop=True,
                        )
                        pbs[(h, 2)] = pb2
                    for h in heads:
                        if (h + qb) % 2 == 0:
                            nc.vector.tensor_copy(A4_tiles[h][:, qb, 0:S], pbs[h])
                            nc.scalar.activation(
                                A4_tiles[h][:, qb, S:NJ],
                                pbs[(h, 2)][:, 0:P],
                                AFT.Identity,
                            )
                        else:
                            nc.scalar.activation(
                                A4_tiles[h][:, qb, 0:S], pbs[h], AFT.Identity
                            )
                            nc.vector.tensor_copy(
                                A4_tiles[h][:, qb, S:NJ],
                                pbs[(h, 2)][:, 0:P],
                            )

                def emit_bias_flush(hp):
                    # after all 4 q-blocks: DRAM roundtrip through the
                    # shift-read, one head per hardware DMA queue
                    for h in (2 * hp, 2 * hp + 1):
                        scr = scrp.tile(
                            [P, TB * NJ], F8, tag="scr", name=f"scr_{h}"
                        )
                        wq_ = nc.sync if h % 2 == 0 else nc.scalar
                        wq_.dma_start(
                            scr, A4_tiles[h].rearrange("p t c -> p (t c)")
                        )
                        # B[qb][p, k] = scr_flat[p*(TB*NJ) + qb*NJ + OFF + k - p]
                        Bt = Bpool.tile([P, TB, S], F8, tag="B", name=f"B_{h}")
                        shifted = bass.AP(
                            scr.tensor, OFF,
                            [[TB * NJ - 1, P], [NJ, TB], [1, S]],
                        )
                        rq_ = nc.scalar if h % 2 == 0 else nc.sync
                        rq_.dma_start(Bt, shifted)
                        B_tiles[h] = Bt

                def emit_attn_pair(hp):
                    heads = (2 * hp, 2 * hp + 1)
                    ex = {
                        h: expool.tile(
                            [P, TB, S], F16, tag="ex", name=f"ex_{h}"
                        )
                        for h in heads
                    }
                    for kb in range(TB):
                        scs = {}
                        for h in heads:
                            sc = psm.tile(
                                [P, S], F32, tag="m", name=f"sc_{h}_{kb}"
                            )
                            nc.tensor.matmul(
                                sc,
                                k_head(h)[:, kb * P : (kb + 1) * P],
                                q_head(h),
                                start=True, stop=False,
                            )
                            scs[h] = sc
                        for qb in range(TB):
                            for h in heads:
                                nc.tensor.matmul(
                                    scs[h][:, qb * P : (qb + 1) * P],
                                    B_tiles[h][:, qb, kb * P : (kb + 1) * P],
                                    idh_sb,
                                    start=False, stop=(qb == TB - 1),
                                    skip_group_check=True,
                                )
                        for h in heads:
                            nc.scalar.activation(ex[h][:, kb, :], scs[h], AFT.Exp)
                    # context + denominator (ones col) in one accumulation
                    for h in heads:
                        ctx = psm.tile([P, S], F32, tag="m", name=f"ctx_{h}")
                        for kb in range(TB):
                            nc.tensor.matmul(
                                ctx[0 : HD + 1, :],
                                V_sb[:, kb, 65 * h : 65 * h + HD + 1],
                                ex[h][:, kb, :],
                                start=(kb == 0), stop=(kb == TB - 1),
                            )
                        den_sb = smallp.tile(
                            [1, S], F32, tag="den", name=f"den_sb_{h}"
                        )
                        nc.vector.tensor_copy(den_sb, ctx[HD : HD + 1, :])
                        rcp = smallp.tile([1, S], F32, tag="rcp", name=f"rcp_{h}")
                        nc.vector.reciprocal_approx_fast(out=rcp, in_=den_sb)
                        dbc = smallp.tile([64, S], F32, tag="dbc", name=f"dbc_{h}")
                        nc.gpsimd.partition_broadcast(dbc, rcp)
                        nc.vector.tensor_mul(
                            ctxT_sb[64 * (h % 2) : 64 * (h % 2) + 64, h // 2, :],
                            ctx[0:HD, :],
                            dbc,
                        )
                        del B_tiles[h]

                # bias (pair, qb) blocks spread across the Q/K/V projection
                # loops so the eviction casts always drain before the PE
                # returns for the next block (no PSUM-slot stalls).
                bias_sched = {
                    # after Q-proj hb=i
                    ("q", 0): [(0, 0)],
                    ("q", 1): [(0, 1), (0, 2)],
                    ("q", 2): [(0, 3), (1, 0)],
                    ("q", 3): [(1, 1), (1, 2)],
                    ("q", 4): [(1, 3), (2, 0)],
                    ("q", 5): [(2, 1), (2, 2)],
                    # after K-proj hb=i
                    ("k", 0): [(2, 3), (3, 0)],
                    ("k", 1): [(3, 1), (3, 2)],
                    ("k", 2): [(3, 3), (4, 0)],
                    ("k", 3): [(4, 1), (4, 2)],
                    ("k", 4): [(4, 3), (5, 0)],
                    ("k", 5): [(5, 1), (5, 2)],
                    # during V projection
                    ("v", 0): [(5, 3)],
                }

                def emit_bias_at(key):
                    for hp, qb in bias_sched.get(key, []):
                        emit_bias_block(hp, qb)
                        if qb == TB - 1:
                            emit_bias_flush(hp)

                # ---- Q projection ----
                for hb in range(HB):
                    psq = psm.tile([P, S], F32, tag="m", name=f"psq_{hb}")
                    for kb in range(HB):
                        nc.tensor.matmul(
                            psq, wq_sb[(kb, hb)],
                            xT_sb[:, kb, :],
                            start=(kb == 0), stop=(kb == HB - 1),
                        )
                    nc.scalar.activation(
                        QT_sb[:, hb, :], psq, AFT.Identity,
                        bias=bq8_sb[:, hb : hb + 1], scale=0.125,
                    )
                    emit_bias_at(("q", hb))

                # ---- K projection ----
                for hb in range(HB):
                    psk = psm.tile([P, S], F32, tag="m", name=f"psk_{hb}")
                    for kb in range(HB):
                        nc.tensor.matmul(
                            psk, wk_sb[kb][:, hb * P : (hb + 1) * P],
                            xT_sb[:, kb, :],
                            start=(kb == 0), stop=(kb == HB - 1),
                        )
                    nc.scalar.activation(
                        KT_sb[:, hb, :], psk, AFT.Identity,
                        bias=bk_sb[:, hb : hb + 1], scale=1.0,
                    )
                    emit_bias_at(("k", hb))

                # ---- V projection ----
                for tb in range(TB):
                    if tb == 0:
                        emit_bias_at(("v", 0))
                    for hf in range(2):
                        psv = psh.tile([P, 384], F32, tag="h", name=f"psv_{tb}_{hf}")
                        for kb in range(HB):
                            nc.tensor.matmul(
                                psv,
                                xT_sb[:, kb, tb * P : (tb + 1) * P],
                                wv_sb[kb][:, hf * 384 : (hf + 1) * 384],
                                start=(kb == 0), stop=(kb == HB - 1),
                            )
                        # scatter 6 heads x 64 cols into the 65-strided layout
                        nc.vector.tensor_copy(
                            V_sb[:, tb, :]
                            .rearrange("p (nh c) -> p nh c", nh=NH, c=HD + 1)[
                                :, 6 * hf : 6 * hf + 6, 0:HD
                            ],
                            psv.rearrange("p (nh c) -> p nh c", nh=6, c=HD),
                        )

                # residual input (with bo+bv@Wo folded in on host) + the
                # later-phase weights; queue order keeps attention unblocked.
                nc.scalar.dma_start(rows_sb, d_rows.ap())
                nc.scalar.dma_start(onesr_sb, d_onesr.ap())
                nc.scalar.dma_start(idf_sb, d_idf.ap())
                nc.sync.dma_start(
                    x_sb, d_x.ap().rearrange("(tb p) h -> p tb h", p=P)
                )
                wo_t = wr_pool.tile([P, HB, H], F16, tag="s2", name="wo_t", bufs=1)
                nc.sync.dma_start(wo_t, d_wo.ap())
                w1_t = wr_pool.tile([P, HB, FF], F16, tag="s3", name="w1_t", bufs=1)
                nc.gpsimd.dma_start(w1_t, d_w1.ap())
                w2_t1 = wr_pool.tile(
                    [P, FB, 384], F16, tag="s1", name="w2_t1", bufs=1
                )
                nc.gpsimd.dma_start(
                    w2_t1, d_w2.ap()[:, :, 384:768]
                )
                wo_sb = [wo_t[:, kb, :] for kb in range(HB)]

                for hp in range(NH // 2):
                    emit_attn_pair(hp)

                # ---- attention output projection: kb-outer in two 4-bank
                # passes so the last head pair's normalize chain is hidden.
                ao_sb = {}
                for tb in (2, 3, 0, 1):
                    ao_sb[tb] = evp.tile([P, H], F32, tag="ao", name=f"ao_{tb}")
                for grp in ((2, 3), (0, 1)):
                    paos = {}
                    for tb in grp:
                        for hf in range(2):
                            paos[(tb, hf)] = psm.tile(
                                [P, 384], F32, tag="m", name=f"pao_{tb}_{hf}"
                            )
                    for kb in range(HB):
                        for tb in grp:
                            for hf in range(2):
                                nc.tensor.matmul(
                                    paos[(tb, hf)],
                                    ctxT_sb[:, kb, tb * P : (tb + 1) * P],
                                    wo_sb[kb][:, hf * 384 : (hf + 1) * 384],
                                    start=(kb == 0), stop=(kb == HB - 1),
                                )
                    for tb in grp:
                        for hf in range(2):
                            nc.vector.tensor_add(
                                ao_sb[tb][:, hf * 384 : (hf + 1) * 384],
                                paos[(tb, hf)],
                                x_sb[:, tb, hf * 384 : (hf + 1) * 384],
                            )
                        # LN1 (scale/bias folded into W1/b1; h1 = normalized)
                        st = statp.tile([P, 2, 6], F32, tag="st", name=f"st1_{tb}")
                        nc.vector.bn_stats(st[:, 0, :], ao_sb[tb][:, 0:384])
                        nc.vector.bn_stats(st[:, 1, :], ao_sb[tb][:, 384:768])
                        ag = statp.tile([P, 2], F32, tag="ag", name=f"ag1_{tb}")
                        nc.vector.bn_aggr(ag, st)
                        sq = statp.tile([P, 1], F32, tag="sq", name=f"sq1_{tb}")
                        nc.scalar.activation(sq, ag[:, 1:2], AFT.Sqrt, bias=eps_sb)
                        rstd = statp.tile([P, 1], F32, tag="rstd", name=f"rstd1_{tb}")
                        nc.vector.reciprocal(rstd, sq)
                        if trivial_ln1:
                            nc.vector.tensor_scalar(
                                h1_sb[:, tb, :], ao_sb[tb], ag[:, 0:1], rstd,
                                ALU.subtract, ALU.mult,
                            )
                        else:
                            nc.vector.tensor_scalar(
                                h1n_sb[:, tb, :], ao_sb[tb], ag[:, 0:1], rstd,
                                ALU.subtract, ALU.mult,
                            )
                            nc.vector.tensor_mul(
                                h1_sb[:, tb, :], h1n_sb[:, tb, :], l1s_sb
                            )
                            nc.vector.tensor_add(
                                h1_sb[:, tb, :], h1_sb[:, tb, :], l1b_sb
                            )

                # transpose LN1-normalized hidden -> feature-major for FFN.
                # tb 2,3 first (their LN1 completes first), then tb 0,1.
                tsrc = h1_sb if trivial_ln1 else h1n_sb
                pts = {}
                for hbg in ((0, 1, 2), (3, 4, 5)):
                    for hb in hbg:
                        pts[hb] = psm.tile([P, S], F32, tag="m", name=f"pt_{hb}")
                        for tb in (2, 3):
                            nc.tensor.transpose(
                                pts[hb][:, tb * P : (tb + 1) * P],
                                tsrc[:, tb, hb * P : (hb + 1) * P],
                                idf_sb,
                            )
                    for hb in hbg:
                        for tb in (0, 1):
                            nc.tensor.transpose(
                                pts[hb][:, tb * P : (tb + 1) * P],
                                tsrc[:, tb, hb * P : (hb + 1) * P],
                                idf_sb,
                            )
                        nc.vector.tensor_copy(h1T_sb[:, hb, :], pts[hb])

            # ================= FFN scope =================
            with (
                tc.tile_pool(name="gpool", bufs=FB) as gpool,
                tc.tile_pool(name="ypool", bufs=1) as ypool,
            ):
                y_sb = ypool.tile([P, TB, H], F32, name="y_sb")
                g_tiles = []
                w1_sb = [w1_t[:, kb, :] for kb in range(HB)]
                # second W2 half-tile: issued on gpsimd (idle during FFN)
                w2_t0 = wr_pool.tile(
                    [P, FB, 384], F16, tag="s2", name="w2_t0", bufs=1
                )
                nc.gpsimd.dma_start(w2_t0, d_w2.ap()[:, :, 0:384])
                w2_tiles = {
                    0: [w2_t0[:, f, :] for f in range(FB)],
                    1: [w2_t1[:, f, :] for f in range(FB)],
                }
                for f in range(FB):
                    pf = psm.tile([P, S], F32, tag="m", name=f"pf_{f}")
                    for kb in range(HB):
                        nc.tensor.matmul(
                            pf, w1_sb[kb][:, f * P : (f + 1) * P],
                            h1T_sb[:, kb, :],
                            start=(kb == 0), stop=(kb == HB - 1),
                        )
                    g = gpool.tile([P, S], F16, tag="g", name=f"g_{f}")
                    nc.scalar.activation(
                        g, pf, AFT.Gelu, bias=b1_sb[:, f : f + 1]
                    )
                    g_tiles.append(g)

                # FFN2 tb-outer: LN2 + output DMA pipeline per token block.
                # hf=1 first (its W2 half loads early on the bulk queue);
                # bn_stats per half right after each half's residual add.
                for tb in range(TB):
                    st = statp.tile([P, 2, 6], F32, tag="st", name=f"st2_{tb}")
                    for hf in (1, 0):
                        py = psh.tile(
                            [P, 384], F32, tag="h", name=f"py_{tb}_{hf}"
                        )
                        nc.tensor.matmul(
                            py, onesr_sb, b2_sb[:, hf * 384 : (hf + 1) * 384],
                            start=True, stop=False,
                        )
                        for f in range(FB):
                            nc.tensor.matmul(
                                py,
                                g_tiles[f][:, tb * P : (tb + 1) * P],
                                w2_tiles[hf][f],
                                start=False, stop=(f == FB - 1),
                                skip_group_check=True,
                            )
                        nc.vector.tensor_add(
                            y_sb[:, tb, hf * 384 : (hf + 1) * 384],
                            py,
                            h1_sb[:, tb, hf * 384 : (hf + 1) * 384],
                        )
                        nc.vector.bn_stats(
                            st[:, 1 - hf, :],
                            y_sb[:, tb, hf * 384 : (hf + 1) * 384],
                        )

                    # LN2 -> output, immediately per token block
                    ag = statp.tile([P, 2], F32, tag="ag", name=f"ag2_{tb}")
                    nc.vector.bn_aggr(ag, st)
                    sq = statp.tile([P, 1], F32, tag="sq", name=f"sq2_{tb}")
                    nc.scalar.activation(sq, ag[:, 1:2], AFT.Sqrt, bias=eps_sb)
                    rstd = statp.tile([P, 1], F32, tag="rstd", name=f"rstd2_{tb}")
                    nc.vector.reciprocal(rstd, sq)
                    o_sb = evp.tile([P, H], F32, tag="o", name=f"o_{tb}")
                    for hf in range(2):
                        hs = slice(hf * 384, (hf + 1) * 384)
                        nc.vector.tensor_scalar(
                            o_sb[:, hs], y_sb[:, tb, hs], ag[:, 0:1], rstd,
                            ALU.subtract, ALU.mult,
                        )
                        if not trivial_ln2:
                            nc.vector.tensor_mul(
                                o_sb[:, hs], o_sb[:, hs], l2s_sb[:, hs]
                            )
                            nc.vector.tensor_add(
                                o_sb[:, hs], o_sb[:, hs], l2b_sb[:, hs]
                            )
                        oq = nc.sync if hf == 0 else nc.scalar
                        oq.dma_start(
                            d_out.ap()[tb * P : (tb + 1) * P, hs], o_sb[:, hs]
                        )

    nc.compile()
    return nc


_CACHE = {}


def _get_nc(trivial_ln1, trivial_ln2):
    key = (trivial_ln1, trivial_ln2)
    if key not in _CACHE:
        _CACHE[key] = build(trivial_ln1, trivial_ln2)
    return _CACHE[key]


def _prepare(inputs):
    f32 = np.float32
    f16 = np.float16
    f8 = ml_dtypes.float8_e4m3fn
    x = np.asarray(inputs["hidden_states"], f32)            # [B, S, H]
    mask = np.asarray(inputs["attention_mask"])
    assert mask.all(), "kernel assumes an all-true attention mask"
    Wq = np.asarray(inputs["Wq"], f32)
    bq = np.asarray(inputs["bq"], f32)
    Wk = np.asarray(inputs["Wk"], f32)
    bk = np.asarray(inputs["bk"], f32)
    Wv = np.asarray(inputs["Wv"], f32)
    bv = np.asarray(inputs["bv"], f32)
    Wo = np.asarray(inputs["Wo"], f32)
    bo = np.asarray(inputs["bo"], f32)
    rel = np.asarray(inputs["rel_table"], f32)              # [1023, 64]
    l1s = np.asarray(inputs["ln1_scale"], f32)
    l1b = np.asarray(inputs["ln1_bias"], f32)
    W1 = np.asarray(inputs["W1"], f32)
    b1 = np.asarray(inputs["b1"], f32)
    W2 = np.asarray(inputs["W2"], f32)
    b2 = np.asarray(inputs["b2"], f32)
    l2s = np.asarray(inputs["ln2_scale"], f32)
    l2b = np.asarray(inputs["ln2_bias"], f32)

    B = x.shape[0]
    trivial_ln1 = bool(np.all(l1s == 1.0) and np.all(l1b == 0.0))
    trivial_ln2 = bool(np.all(l2s == 1.0) and np.all(l2b == 0.0))

    # host-side folds (exact algebra)
    bo_p = bo + bv @ Wo                      # V-bias folded via softmax row-sum
    RT = np.zeros((P, 1024), f16)
    RT[:HD, :1023] = (8.0 * rel[::-1].T).astype(f16)  # Q pre-scaled by 1/8
    RT[HD:] = RT[:HD]   # duplicated so odd heads (partitions 64:128) match
    W1f = l1s[:, None] * W1
    b1f = b1 + l1b @ W1

    def tile_rows(W, dt=f16):
        # [K, M] -> [128, K//128, M] so the DMA is a single contiguous copy
        K, M = W.shape
        return np.ascontiguousarray(
            W.astype(dt).reshape(K // P, P, M).transpose(1, 0, 2)
        )

    bcombo = np.empty((P, 36), f32)
    bcombo[:, 0:HB] = (bq / 8.0).reshape(HB, P).T
    bcombo[:, HB : 2 * HB] = bk.reshape(HB, P).T
    bcombo[:, 2 * HB :] = b1f.reshape(FB, P).T

    # wq in [P, hb_out, kb, 128] order: the first two output blocks form a
    # contiguous chunk, so Q's first matmuls can start on a partial load
    wq_hb = np.ascontiguousarray(
        Wq.astype(f16).reshape(HB, P, HB, P).transpose(1, 2, 0, 3)
    )
    common = {
        "wq": wq_hb,
        "wk": tile_rows(Wk),
        "wv": tile_rows(Wv),
        "wo": tile_rows(Wo),
        "w1": tile_rows(W1f),
        "w2": tile_rows(W2),
        "rt": RT,
        "bcombo": bcombo,
        "rows2": b2.astype(f16)[None, :],
        "ones_row": np.ones((1, P), f16),
        "ident_f8": np.eye(P, dtype=f8),
        "ident_f32": np.eye(P, dtype=f32),
    }
    if not trivial_ln1:
        common["ln1s_b"] = np.broadcast_to(l1s, (P, H)).copy()
        common["ln1b_b"] = np.broadcast_to(l1b, (P, H)).copy()
    if not trivial_ln2:
        common["ln2s_b"] = np.broadcast_to(l2s, (P, H)).copy()
        common["ln2b_b"] = np.broadcast_to(l2b, (P, H)).copy()

    in_maps = []
    for b in range(B):
        m = dict(common)
        m["xT"] = tile_rows(x[b].T)
        m["x_res"] = (x[b] + bo_p[None, :]).astype(f16)
        in_maps.append(m)
    return in_maps, trivial_ln1, trivial_ln2, x.dtype


def run(inputs, trace=False, **kw):
    in_maps, t1, t2, dt = _prepare(inputs)
    nc = _get_nc(t1, t2)
    last_err = None
    for attempt in range(3):
        try:
            res = run_bass_kernel_spmd(
                nc, in_maps, core_ids=list(range(len(in_maps))),
                trace=trace, **kw,
            )
            break
        except Exception as e:  # transient NRT_EXEC_UNIT_UNRECOVERABLE etc.
            last_err = e
            import time as _time

            _time.sleep(10)
    else:
        raise last_err
    out = np.stack([res.results[c]["out"] for c in range(len(in_maps))])
    return out.astype(dt, copy=False), res


def kernel(**inputs) -> np.ndarray:
    out, _ = run(inputs, trace=False)
    return out


# revision 24
# speedup vs baseline: 1.0188x; 1.0188x over previous
"""Trainium2 Bass kernel for a BERT layer with relative-position attention bias.

Contract: kernel(**inputs) takes the FULL inputs (as produced by the problem's
setup_inputs) and returns the FULL output [8, 512, 768] float32.

Strategy: data-parallel over batch (B=8 -> one batch element per NeuronCore),
weights replicated, no collectives. Per-core dataflow:

  - activations kept feature-major ([H, S]) for Q/K and the FFN intermediate,
    token-major ([S, H]) for V / attn-out / layernorms.
  - scores computed k-major (scoresT[k, q]) so softmax normalization is a
    per-head partition-broadcast multiply and the context matmul consumes
    exp(scores) directly (no probs transpose).
  - relative-position bias via the Toeplitz/shift trick: per (head, q-block)
    A = Q_blk^T @ RT window [128, 640] -> DRAM (fp8) -> shifted strided DMA
    read back as B[q, k] [128, 512] -> transposed-accumulated into the scores
    PSUM with identity matmuls.  All 12 heads' bias pipelines run during the
    Q/K projections; the 12 fp8 B tiles stay resident so attention never
    stalls on the DRAM roundtrip.
  - softmax denominator accumulated by the context matmul itself via an
    interleaved ones-column in V (65 columns per head).
  - softmax without max-subtraction (scores are O(1); same math).
  - matmuls in fp16 (fast weight loads, ~3e-4 matmul rel err); fp32
    accumulation in PSUM, fp32 layernorm/residual arithmetic.
  - Wo accumulated kb-outer in two 4-bank passes so the last head pair's
    normalize chain is hidden; FFN2 runs tb-outer so LN2 + output DMA
    pipeline with the remaining matmuls.
"""
import os
import sys

for _p in ("/opt/trn_rl_repo", os.path.expanduser("~/.axon_site/_ro/trn_rl_repo")):
    if os.path.isdir(_p) and _p not in sys.path:
        sys.path.insert(0, _p)

import numpy as np
import ml_dtypes

import concourse.bass as bass
import concourse.mybir as mybir
import concourse.tile as tile
from concourse import bacc
from concourse.bass_utils import run_bass_kernel_spmd

P = 128
S = 512
H = 768
NH = 12
HD = 64
FF = 3072
MAXPOS = 512
EPS = 1e-12
HB = H // P       # 6 feature blocks
TB = S // P       # 4 token blocks
FB = FF // P      # 24 ff blocks
NJ = 640          # rel window width per q-block
OFF = 127         # shift-read column offset
VW = NH * (HD + 1)  # V row width: 12 heads x (64 value cols + 1 ones col)

F32 = mybir.dt.float32
F16 = mybir.dt.float16
F8 = mybir.dt.float8e4

AFT = mybir.ActivationFunctionType
ALU = mybir.AluOpType


def build(trivial_ln1: bool, trivial_ln2: bool):
    nc = bacc.Bacc("TRN2", target_bir_lowering=False, debug=False)

    # ---------------- DRAM I/O ----------------
    d_xT = nc.dram_tensor("xT", [P, HB, S], F16, kind="ExternalInput")
    d_x = nc.dram_tensor("x_res", [S, H], F16, kind="ExternalInput")
    d_wq = nc.dram_tensor("wq", [P, HB, HB, P], F16, kind="ExternalInput")
    d_wk = nc.dram_tensor("wk", [P, HB, H], F16, kind="ExternalInput")
    d_wv = nc.dram_tensor("wv", [P, HB, H], F16, kind="ExternalInput")
    d_wo = nc.dram_tensor("wo", [P, HB, H], F16, kind="ExternalInput")
    d_w1 = nc.dram_tensor("w1", [P, HB, FF], F16, kind="ExternalInput")
    d_w2 = nc.dram_tensor("w2", [P, FB, H], F16, kind="ExternalInput")
    d_rt = nc.dram_tensor("rt", [P, 1024], F16, kind="ExternalInput")
    d_bc = nc.dram_tensor("bcombo", [P, 36], F32, kind="ExternalInput")
    d_rows = nc.dram_tensor("rows2", [1, H], F16, kind="ExternalInput")
    d_onesr = nc.dram_tensor("ones_row", [1, P], F16, kind="ExternalInput")
    d_idh = nc.dram_tensor("ident_f8", [P, P], F8, kind="ExternalInput")
    d_idf = nc.dram_tensor("ident_f32", [P, P], F32, kind="ExternalInput")
    if not trivial_ln1:
        d_l1s = nc.dram_tensor("ln1s_b", [P, H], F32, kind="ExternalInput")
        d_l1b = nc.dram_tensor("ln1b_b", [P, H], F32, kind="ExternalInput")
    if not trivial_ln2:
        d_l2s = nc.dram_tensor("ln2s_b", [P, H], F32, kind="ExternalInput")
        d_l2b = nc.dram_tensor("ln2b_b", [P, H], F32, kind="ExternalInput")
    d_out = nc.dram_tensor("out", [S, H], F32, kind="ExternalOutput")

    with tile.TileContext(nc) as tc:
        with (
            tc.tile_pool(name="const", bufs=1) as const,
            tc.tile_pool(name="persist", bufs=1) as persist,
            tc.tile_pool(name="wr", bufs=4) as wr_pool,
            tc.tile_pool(name="psm", bufs=5, space="PSUM") as psm,
            tc.tile_pool(name="psh", bufs=3, space="PSUM") as psh,
            tc.tile_pool(name="stat", bufs=4) as statp,
            tc.tile_pool(name="evict", bufs=2) as evp,
        ):
            # ---- PE warm-up: junk matmuls release the HAM clock throttle
            # while the first DMAs are still in flight.
            warm_w = const.tile([P, P], F16, name="warm_w")
            nc.gpsimd.memset(warm_w, 0.0)
            for wi in range(32):
                pw = psm.tile([P, P], F32, tag="m", name=f"warm_{wi}")
                nc.tensor.matmul(pw, warm_w, warm_w, start=True, stop=True)

            # ---- input + first-use weights.  Three DMA queues (sync=q1,
            # scalar=q10, gpsimd=q0); balance the Q-critical bytes across
            # sync+scalar with big descriptors (wq is host-tiled hb-major so
            # the first 2 output blocks arrive as their own chunk); gpsimd's
            # software-DGE queue carries the bulk low-urgency weights.
            xT_sb = persist.tile([P, HB, S], F16, name="xT_sb")
            wq_t = wr_pool.tile([P, HB, HB, P], F16, tag="s1", name="wq_t", bufs=1)
            wk_t = wr_pool.tile([P, HB, H], F16, tag="s2", name="wk_t", bufs=1)
            wv_t = wr_pool.tile([P, HB, H], F16, tag="s3", name="wv_t", bufs=1)
            nc.sync.dma_start(xT_sb[:, 0:3, :], d_xT.ap()[:, 0:3, :])
            nc.gpsimd.dma_start(xT_sb[:, 3:HB, :], d_xT.ap()[:, 3:HB, :])
            nc.scalar.dma_start(wq_t[:, 0:2], d_wq.ap()[:, 0:2])
            nc.sync.dma_start(wq_t[:, 2:HB], d_wq.ap()[:, 2:HB])
            # rt next on scalar (needed by the first bias matmuls ~13us)
            rt_sb = const.tile([P, 1024], F16, name="rt_sb")
            nc.scalar.dma_start(rt_sb, d_rt.ap())
            bc_sb = const.tile([P, 36], F32, name="bc_sb")
            nc.scalar.dma_start(bc_sb, d_bc.ap())
            idh_sb = const.tile([P, P], F8, name="idh_sb")
            nc.scalar.dma_start(idh_sb, d_idh.ap())
            nc.sync.dma_start(wk_t, d_wk.ap())
            nc.scalar.dma_start(wv_t, d_wv.ap())
            wq_sb = {(kb, hb): wq_t[:, hb, kb, :] for kb in range(HB) for hb in range(HB)}
            wk_sb = [wk_t[:, kb, :] for kb in range(HB)]
            wv_sb = [wv_t[:, kb, :] for kb in range(HB)]

            bq8_sb = bc_sb[:, 0:HB]
            bk_sb = bc_sb[:, HB : 2 * HB]
            b1_sb = bc_sb[:, 2 * HB : 2 * HB + FB]
            # low-urgency consts (transpose identity, FFN bias rows)
            rows_sb = const.tile([1, H], F16, name="rows_sb")
            onesr_sb = const.tile([1, P], F16, name="onesr_sb")
            idf_sb = const.tile([P, P], F32, name="idf_sb")
            b2_sb = rows_sb[:, 0:H]
            eps_sb = const.tile([P, 1], F32, name="eps_sb")
            nc.gpsimd.memset(eps_sb, EPS)
            if not trivial_ln1:
                l1s_sb = const.tile([P, H], F32, name="l1s_sb")
                nc.scalar.dma_start(l1s_sb, d_l1s.ap())
                l1b_sb = const.tile([P, H], F32, name="l1b_sb")
                nc.scalar.dma_start(l1b_sb, d_l1b.ap())
            if not trivial_ln2:
                l2s_sb = const.tile([P, H], F32, name="l2s_sb")
                nc.scalar.dma_start(l2s_sb, d_l2s.ap())
                l2b_sb = const.tile([P, H], F32, name="l2b_sb")
                nc.scalar.dma_start(l2b_sb, d_l2b.ap())

            # ---- persistent activations ----
            h1_sb = persist.tile([P, TB, H], F32, name="h1_sb")
            h1T_sb = persist.tile([P, HB, S], F16, name="h1T_sb")
            if not trivial_ln1:
                h1n_sb = persist.tile([P, TB, H], F32, name="h1n_sb")

            # ================= attention scope =================
            with (
                tc.tile_pool(name="attn", bufs=1) as ap_,
                tc.tile_pool(name="expool", bufs=4) as expool,
                tc.tile_pool(name="Apool", bufs=4) as Apool,
                tc.tile_pool(name="Bpool", bufs=10) as Bpool,
                tc.tile_pool(name="smallp", bufs=2) as smallp,
                tc.tile_pool(name="scr", bufs=12, space="DRAM") as scrp,
            ):
                x_sb = ap_.tile([P, TB, H], F16, name="x_sb")
                QT_sb = ap_.tile([P, HB, S], F16, name="QT_sb")
                KT_sb = ap_.tile([P, HB, S], F16, name="KT_sb")
                # V with an interleaved ones-column per head: head h occupies
                # columns [65h, 65h+64), column 65h+64 is ones so the context
                # matmul also produces the softmax denominator in row 64.
                V_sb = ap_.tile([P, TB, VW], F8, name="V_sb")
                nc.vector.memset(V_sb, 1.0)
                ctxT_sb = ap_.tile([P, HB, S], F16, name="ctxT_sb")

                def q_head(h):
                    return QT_sb[64 * (h % 2) : 64 * (h % 2) + 64, h // 2, :]

                def k_head(h):
                    return KT_sb[64 * (h % 2) : 64 * (h % 2) + 64, h // 2, :]

                B_tiles = {}
                A4_tiles = {}

                def emit_bias_block(hp, qb):
                    # one (head-pair, q-block) slice of the rel-bias pipeline:
                    # 3 PSUM tiles (pb1 x2 heads, shared pb2), 3 matmuls,
                    # 4 eviction casts split across vector + scalar.
                    heads = (2 * hp, 2 * hp + 1)
                    if qb == 0:
                        for h in heads:
                            A4_tiles[h] = Apool.tile(
                                [P, TB, NJ], F8, tag="A", name=f"A_{h}"
                            )
                    q0 = qb * P
                    j0 = 384 - q0
                    pbs = {}
                    for h in heads:
                        Qh = q_head(h)
                        b0 = 64 * (h % 2)
                        rth = rt_sb[b0 : b0 + HD, :]
                        pb1 = psh.tile(
                            [P, S], F32, tag="h", name=f"pb1_{h}_{qb}"
                        )
                        nc.tensor.matmul(
                            pb1, Qh[:, q0 : q0 + P], rth[:, j0 : j0 + S],
                            start=True, stop=True,
                        )
                        pbs[h] = pb1
                    for h in heads:
                        Qh = q_head(h)
                        b0 = 64 * (h % 2)
                        rth = rt_sb[b0 : b0 + HD, :]
                        pb2 = psh.tile(
                            [P, S], F32, tag="h", name=f"pb2_{h}_{qb}"
                        )
                        nc.tensor.matmul(
                            pb2[:, 0:P], Qh[:, q0 : q0 + P],
                            rth[:, j0 + S : j0 + NJ],
                            start=True, stop=True,
                        )
                        pbs[(h, 2)] = pb2
                    for h in heads:
                        if (h + qb) % 2 == 0:
                            nc.vector.tensor_copy(A4_tiles[h][:, qb, 0:S], pbs[h])
                            nc.scalar.activation(
                                A4_tiles[h][:, qb, S:NJ],
                                pbs[(h, 2)][:, 0:P],
                                AFT.Identity,
                            )
                        else:
                            nc.scalar.activation(
                                A4_tiles[h][:, qb, 0:S], pbs[h], AFT.Identity
                            )
                            nc.vector.tensor_copy(
                                A4_tiles[h][:, qb, S:NJ],
                                pbs[(h, 2)][:, 0:P],
                            )

                def emit_bias_flush(hp):
                    # after all 4 q-blocks: DRAM roundtrip through the
                    # shift-read, one head per hardware DMA queue
                    for h in (2 * hp, 2 * hp + 1):
                        scr = scrp.tile(
                            [P, TB * NJ], F8, tag="scr", name=f"scr_{h}"
                        )
                        wq_ = nc.sync if h % 2 == 0 else nc.scalar
                        wq_.dma_start(
                            scr, A4_tiles[h].rearrange("p t c -> p (t c)")
                        )
                        # B[qb][p, k] = scr_flat[p*(TB*NJ) + qb*NJ + OFF + k - p]
                        Bt = Bpool.tile([P, TB, S], F8, tag="B", name=f"B_{h}")
                        shifted = bass.AP(
                            scr.tensor, OFF,
                            [[TB * NJ - 1, P], [NJ, TB], [1, S]],
                        )
                        rq_ = nc.scalar if h % 2 == 0 else nc.sync
                        rq_.dma_start(Bt, shifted)
                        B_tiles[h] = Bt

                ex_tiles = {}

                def emit_scores_kb(hp, kb):
                    heads = (2 * hp, 2 * hp + 1)
                    if kb == 0:
                        for h in heads:
                            ex_tiles[h] = expool.tile(
                                [P, TB, S], F8, tag="ex", name=f"ex_{h}"
                            )
                    scs = {}
                    for h in heads:
                        sc = psm.tile(
                            [P, S], F32, tag="m", name=f"sc_{h}_{kb}"
                        )
                        nc.tensor.matmul(
                            sc,
                            k_head(h)[:, kb * P : (kb + 1) * P],
                            q_head(h),
                            start=True, stop=False,
                        )
                        scs[h] = sc
                    for qb in range(TB):
                        for h in heads:
                            nc.tensor.matmul(
                                scs[h][:, qb * P : (qb + 1) * P],
                                B_tiles[h][:, qb, kb * P : (kb + 1) * P],
                                idh_sb,
                                start=False, stop=(qb == TB - 1),
                                skip_group_check=True,
                            )
                    for h in heads:
                        nc.scalar.activation(
                            ex_tiles[h][:, kb, :], scs[h], AFT.Exp
                        )

                def emit_ctx_head(h):
                    # context + denominator (ones col) in one accumulation
                    ex = ex_tiles[h]
                    ctx = psm.tile([P, S], F32, tag="m", name=f"ctx_{h}")
                    for kb in range(TB):
                        nc.tensor.matmul(
                            ctx[0 : HD + 1, :],
                            V_sb[:, kb, 65 * h : 65 * h + HD + 1],
                            ex[:, kb, :],
                            start=(kb == 0), stop=(kb == TB - 1),
                        )
                    den_sb = smallp.tile(
                        [1, S], F32, tag="den", name=f"den_sb_{h}"
                    )
                    nc.vector.tensor_copy(den_sb, ctx[HD : HD + 1, :])
                    rcp = smallp.tile([1, S], F32, tag="rcp", name=f"rcp_{h}")
                    nc.vector.reciprocal_approx_fast(out=rcp, in_=den_sb)
                    dbc = smallp.tile([64, S], F32, tag="dbc", name=f"dbc_{h}")
                    nc.gpsimd.partition_broadcast(dbc, rcp)
                    nc.vector.tensor_mul(
                        ctxT_sb[64 * (h % 2) : 64 * (h % 2) + 64, h // 2, :],
                        ctx[0:HD, :],
                        dbc,
                    )
                    del B_tiles[h]
                    del ex_tiles[h]

                # bias (pair, qb) blocks spread across the Q/K/V projection
                # loops so the eviction casts always drain before the PE
                # returns for the next block (no PSUM-slot stalls).
                bias_sched = {
                    # after Q-proj hb=i
                    ("q", 0): [(0, 0)],
                    ("q", 1): [(0, 1), (0, 2)],
                    ("q", 2): [(0, 3), (1, 0)],
                    ("q", 3): [(1, 1), (1, 2)],
                    ("q", 4): [(1, 3), (2, 0)],
                    ("q", 5): [(2, 1), (2, 2)],
                    # after K-proj hb=i
                    ("k", 0): [(2, 3), (3, 0)],
                    ("k", 1): [(3, 1), (3, 2)],
                    ("k", 2): [(3, 3), (4, 0)],
                    ("k", 3): [(4, 1), (4, 2)],
                    ("k", 4): [(4, 3), (5, 0)],
                    ("k", 5): [(5, 1), (5, 2)],
                    # during V projection
                    ("v", 0): [(5, 3)],
                }

                def emit_bias_at(key):
                    for hp, qb in bias_sched.get(key, []):
                        emit_bias_block(hp, qb)
                        if qb == TB - 1:
                            emit_bias_flush(hp)

                # ---- Q projection ----
                for hb in range(HB):
                    psq = psm.tile([P, S], F32, tag="m", name=f"psq_{hb}")
                    for kb in range(HB):
                        nc.tensor.matmul(
                            psq, wq_sb[(kb, hb)],
                            xT_sb[:, kb, :],
                            start=(kb == 0), stop=(kb == HB - 1),
                        )
                    nc.vector.tensor_scalar(
                        QT_sb[:, hb, :], psq, 0.125, bq8_sb[:, hb : hb + 1],
                        ALU.mult, ALU.add,
                    )
                    emit_bias_at(("q", hb))

                # ---- K projection ----
                for hb in range(HB):
                    psk = psm.tile([P, S], F32, tag="m", name=f"psk_{hb}")
                    for kb in range(HB):
                        nc.tensor.matmul(
                            psk, wk_sb[kb][:, hb * P : (hb + 1) * P],
                            xT_sb[:, kb, :],
                            start=(kb == 0), stop=(kb == HB - 1),
                        )
                    nc.vector.tensor_scalar_add(
                        KT_sb[:, hb, :], psk, bk_sb[:, hb : hb + 1]
                    )
                    emit_bias_at(("k", hb))

                # ---- V projection ----
                for tb in range(TB):
                    if tb == 0:
                        emit_bias_at(("v", 0))
                    for hf in range(2):
                        psv = psh.tile([P, 384], F32, tag="h", name=f"psv_{tb}_{hf}")
                        for kb in range(HB):
                            nc.tensor.matmul(
                                psv,
                                xT_sb[:, kb, tb * P : (tb + 1) * P],
                                wv_sb[kb][:, hf * 384 : (hf + 1) * 384],
                                start=(kb == 0), stop=(kb == HB - 1),
                            )
                        # scatter 6 heads x 64 cols into the 65-strided layout
                        nc.vector.tensor_copy(
                            V_sb[:, tb, :]
                            .rearrange("p (nh c) -> p nh c", nh=NH, c=HD + 1)[
                                :, 6 * hf : 6 * hf + 6, 0:HD
                            ],
                            psv.rearrange("p (nh c) -> p nh c", nh=6, c=HD),
                        )

                # residual input (with bo+bv@Wo folded in on host) + the
                # later-phase weights; queue order keeps attention unblocked.
                nc.scalar.dma_start(rows_sb, d_rows.ap())
                nc.scalar.dma_start(onesr_sb, d_onesr.ap())
                nc.scalar.dma_start(idf_sb, d_idf.ap())
                nc.sync.dma_start(
                    x_sb, d_x.ap().rearrange("(tb p) h -> p tb h", p=P)
                )
                wo_t = wr_pool.tile([P, HB, H], F16, tag="s2", name="wo_t", bufs=1)
                nc.sync.dma_start(wo_t, d_wo.ap())
                w1_t = wr_pool.tile([P, HB, FF], F16, tag="s3", name="w1_t", bufs=1)
                nc.gpsimd.dma_start(w1_t, d_w1.ap())
                w2_t1 = wr_pool.tile(
                    [P, FB, 384], F16, tag="s1", name="w2_t1", bufs=1
                )
                nc.gpsimd.dma_start(
                    w2_t1, d_w2.ap()[:, :, 384:768]
                )
                wo_sb = [wo_t[:, kb, :] for kb in range(HB)]

                # attention pairs, software-pipelined: the previous pair's
                # context matmuls slot between this pair's kb-steps so the
                # PE never idles on the exp chain (keeps the HAM clock warm)
                pend = []
                for hp in range(NH // 2):
                    for kb in range(TB):
                        emit_scores_kb(hp, kb)
                        if kb in (1, 3) and pend:
                            emit_ctx_head(pend.pop(0))
                    pend += [2 * hp, 2 * hp + 1]
                for h in pend:
                    emit_ctx_head(h)

                # ---- attention output projection: kb-outer in two 4-bank
                # passes so the last head pair's normalize chain is hidden.
                ao_sb = {}
                for tb in (2, 3, 0, 1):
                    ao_sb[tb] = evp.tile([P, H], F32, tag="ao", name=f"ao_{tb}")
                for grp in ((2, 3), (0, 1)):
                    paos = {}
                    for tb in grp:
                        for hf in range(2):
                            paos[(tb, hf)] = psm.tile(
                                [P, 384], F32, tag="m", name=f"pao_{tb}_{hf}"
                            )
                    for kb in range(HB):
                        for tb in grp:
                            for hf in range(2):
                                nc.tensor.matmul(
                                    paos[(tb, hf)],
                                    ctxT_sb[:, kb, tb * P : (tb + 1) * P],
                                    wo_sb[kb][:, hf * 384 : (hf + 1) * 384],
                                    start=(kb == 0), stop=(kb == HB - 1),
                                )
                    for tb in grp:
                        for hf in range(2):
                            nc.vector.tensor_add(
                                ao_sb[tb][:, hf * 384 : (hf + 1) * 384],
                                paos[(tb, hf)],
                                x_sb[:, tb, hf * 384 : (hf + 1) * 384],
                            )
                        # LN1 (scale/bias folded into W1/b1; h1 = normalized)
                        st = statp.tile([P, 2, 6], F32, tag="st", name=f"st1_{tb}")
                        nc.vector.bn_stats(st[:, 0, :], ao_sb[tb][:, 0:384])
                        nc.vector.bn_stats(st[:, 1, :], ao_sb[tb][:, 384:768])
                        ag = statp.tile([P, 2], F32, tag="ag", name=f"ag1_{tb}")
                        nc.vector.bn_aggr(ag, st)
                        sq = statp.tile([P, 1], F32, tag="sq", name=f"sq1_{tb}")
                        nc.scalar.activation(sq, ag[:, 1:2], AFT.Sqrt, bias=eps_sb)
                        rstd = statp.tile([P, 1], F32, tag="rstd", name=f"rstd1_{tb}")
                        nc.vector.reciprocal(rstd, sq)
                        if trivial_ln1:
                            nc.vector.tensor_scalar(
                                h1_sb[:, tb, :], ao_sb[tb], ag[:, 0:1], rstd,
                                ALU.subtract, ALU.mult,
                            )
                        else:
                            nc.vector.tensor_scalar(
                                h1n_sb[:, tb, :], ao_sb[tb], ag[:, 0:1], rstd,
                                ALU.subtract, ALU.mult,
                            )
                            nc.vector.tensor_mul(
                                h1_sb[:, tb, :], h1n_sb[:, tb, :], l1s_sb
                            )
                            nc.vector.tensor_add(
                                h1_sb[:, tb, :], h1_sb[:, tb, :], l1b_sb
                            )

                # transpose LN1-normalized hidden -> feature-major for FFN.
                # tb 2,3 first (their LN1 completes first), then tb 0,1.
                tsrc = h1_sb if trivial_ln1 else h1n_sb
                pts = {}
                for hbg in ((0, 1, 2), (3, 4, 5)):
                    for hb in hbg:
                        pts[hb] = psm.tile([P, S], F32, tag="m", name=f"pt_{hb}")
                        for tb in (2, 3):
                            nc.tensor.transpose(
                                pts[hb][:, tb * P : (tb + 1) * P],
                                tsrc[:, tb, hb * P : (hb + 1) * P],
                                idf_sb,
                            )
                    for hb in hbg:
                        for tb in (0, 1):
                            nc.tensor.transpose(
                                pts[hb][:, tb * P : (tb + 1) * P],
                                tsrc[:, tb, hb * P : (hb + 1) * P],
                                idf_sb,
                            )
                        nc.vector.tensor_copy(h1T_sb[:, hb, :], pts[hb])

            # ================= FFN scope =================
            with (
                tc.tile_pool(name="gpool", bufs=FB) as gpool,
                tc.tile_pool(name="ypool", bufs=1) as ypool,
            ):
                y_sb = ypool.tile([P, TB, H], F32, name="y_sb")
                g_tiles = []
                w1_sb = [w1_t[:, kb, :] for kb in range(HB)]
                # second W2 half-tile: issued on gpsimd (idle during FFN)
                w2_t0 = wr_pool.tile(
                    [P, FB, 384], F16, tag="s2", name="w2_t0", bufs=1
                )
                nc.gpsimd.dma_start(w2_t0, d_w2.ap()[:, :, 0:384])
                w2_tiles = {
                    0: [w2_t0[:, f, :] for f in range(FB)],
                    1: [w2_t1[:, f, :] for f in range(FB)],
                }
                for f in range(FB):
                    pf = psm.tile([P, S], F32, tag="m", name=f"pf_{f}")
                    for kb in range(HB):
                        nc.tensor.matmul(
                            pf, w1_sb[kb][:, f * P : (f + 1) * P],
                            h1T_sb[:, kb, :],
                            start=(kb == 0), stop=(kb == HB - 1),
                        )
                    g = gpool.tile([P, S], F16, tag="g", name=f"g_{f}")
                    nc.scalar.activation(
                        g, pf, AFT.Gelu, bias=b1_sb[:, f : f + 1]
                    )
                    g_tiles.append(g)

                # FFN2 tb-outer: LN2 + output DMA pipeline per token block.
                # hf=1 first (its W2 half loads early on the bulk queue);
                # bn_stats per half right after each half's residual add.
                for tb in range(TB):
                    st = statp.tile([P, 2, 6], F32, tag="st", name=f"st2_{tb}")
                    for hf in (1, 0):
                        py = psh.tile(
                            [P, 384], F32, tag="h", name=f"py_{tb}_{hf}"
                        )
                        nc.tensor.matmul(
                            py, onesr_sb, b2_sb[:, hf * 384 : (hf + 1) * 384],
                            start=True, stop=False,
                        )
                        for f in range(FB):
                            nc.tensor.matmul(
                                py,
                                g_tiles[f][:, tb * P : (tb + 1) * P],
                                w2_tiles[hf][f],
                                start=False, stop=(f == FB - 1),
                                skip_group_check=True,
                            )
                        nc.vector.tensor_add(
                            y_sb[:, tb, hf * 384 : (hf + 1) * 384],
                            py,
                            h1_sb[:, tb, hf * 384 : (hf + 1) * 384],
                        )
                        nc.vector.bn_stats(
                            st[:, 1 - hf, :],
                            y_sb[:, tb, hf * 384 : (hf + 1) * 384],
                        )

                    # LN2 -> output, immediately per token block
                    ag = statp.tile([P, 2], F32, tag="ag", name=f"ag2_{tb}")
                    nc.vector.bn_aggr(ag, st)
                    sq = statp.tile([P, 1], F32, tag="sq", name=f"sq2_{tb}")
                    nc.scalar.activation(sq, ag[:, 1:2], AFT.Sqrt, bias=eps_sb)
                    rstd = statp.tile([P, 1], F32, tag="rstd", name=f"rstd2_{tb}")
                    nc.vector.reciprocal(rstd, sq)
                    o_sb = evp.tile([P, H], F32, tag="o", name=f"o_{tb}")
                    for hf in range(2):
                        hs = slice(hf * 384, (hf + 1) * 384)
                        nc.vector.tensor_scalar(
                            o_sb[:, hs], y_sb[:, tb, hs], ag[:, 0:1], rstd,
                            ALU.subtract, ALU.mult,
                        )
                        if not trivial_ln2:
                            nc.vector.tensor_mul(
                                o_sb[:, hs], o_sb[:, hs], l2s_sb[:, hs]
                            )
                            nc.vector.tensor_add(
                                o_sb[:, hs], o_sb[:, hs], l2b_sb[:, hs]
                            )
                        oq = nc.sync if hf == 0 else nc.scalar
                        oq.dma_start(
                            d_out.ap()[tb * P : (tb + 1) * P, hs], o_sb[:, hs]
                        )

    nc.compile()
    return nc


_CACHE = {}


def _get_nc(trivial_ln1, trivial_ln2):
    key = (trivial_ln1, trivial_ln2)
    if key not in _CACHE:
        _CACHE[key] = build(trivial_ln1, trivial_ln2)
    return _CACHE[key]


def _prepare(inputs):
    f32 = np.float32
    f16 = np.float16
    f8 = ml_dtypes.float8_e4m3fn
    x = np.asarray(inputs["hidden_states"], f32)            # [B, S, H]
    mask = np.asarray(inputs["attention_mask"])
    assert mask.all(), "kernel assumes an all-true attention mask"
    Wq = np.asarray(inputs["Wq"], f32)
    bq = np.asarray(inputs["bq"], f32)
    Wk = np.asarray(inputs["Wk"], f32)
    bk = np.asarray(inputs["bk"], f32)
    Wv = np.asarray(inputs["Wv"], f32)
    bv = np.asarray(inputs["bv"], f32)
    Wo = np.asarray(inputs["Wo"], f32)
    bo = np.asarray(inputs["bo"], f32)
    rel = np.asarray(inputs["rel_table"], f32)              # [1023, 64]
    l1s = np.asarray(inputs["ln1_scale"], f32)
    l1b = np.asarray(inputs["ln1_bias"], f32)
    W1 = np.asarray(inputs["W1"], f32)
    b1 = np.asarray(inputs["b1"], f32)
    W2 = np.asarray(inputs["W2"], f32)
    b2 = np.asarray(inputs["b2"], f32)
    l2s = np.asarray(inputs["ln2_scale"], f32)
    l2b = np.asarray(inputs["ln2_bias"], f32)

    B = x.shape[0]
    trivial_ln1 = bool(np.all(l1s == 1.0) and np.all(l1b == 0.0))
    trivial_ln2 = bool(np.all(l2s == 1.0) and np.all(l2b == 0.0))

    # host-side folds (exact algebra)
    bo_p = bo + bv @ Wo                      # V-bias folded via softmax row-sum
    RT = np.zeros((P, 1024), f16)
    RT[:HD, :1023] = (8.0 * rel[::-1].T).astype(f16)  # Q pre-scaled by 1/8
    RT[HD:] = RT[:HD]   # duplicated so odd heads (partitions 64:128) match
    W1f = l1s[:, None] * W1
    b1f = b1 + l1b @ W1

    def tile_rows(W, dt=f16):
        # [K, M] -> [128, K//128, M] so the DMA is a single contiguous copy
        K, M = W.shape
        return np.ascontiguousarray(
            W.astype(dt).reshape(K // P, P, M).transpose(1, 0, 2)
        )

    bcombo = np.empty((P, 36), f32)
    bcombo[:, 0:HB] = (bq / 8.0).reshape(HB, P).T
    bcombo[:, HB : 2 * HB] = bk.reshape(HB, P).T
    bcombo[:, 2 * HB :] = b1f.reshape(FB, P).T

    # wq in [P, hb_out, kb, 128] order: the first two output blocks form a
    # contiguous chunk, so Q's first matmuls can start on a partial load
    wq_hb = np.ascontiguousarray(
        Wq.astype(f16).reshape(HB, P, HB, P).transpose(1, 2, 0, 3)
    )
    common = {
        "wq": wq_hb,
        "wk": tile_rows(Wk),
        "wv": tile_rows(Wv),
        "wo": tile_rows(Wo),
        "w1": tile_rows(W1f),
        "w2": tile_rows(W2),
        "rt": RT,
        "bcombo": bcombo,
        "rows2": b2.astype(f16)[None, :],
        "ones_row": np.ones((1, P), f16),
        "ident_f8": np.eye(P, dtype=f8),
        "ident_f32": np.eye(P, dtype=f32),
    }
    if not trivial_ln1:
        common["ln1s_b"] = np.broadcast_to(l1s, (P, H)).copy()
        common["ln1b_b"] = np.broadcast_to(l1b, (P, H)).copy()
    if not trivial_ln2:
        common["ln2s_b"] = np.broadcast_to(l2s, (P, H)).copy()
        common["ln2b_b"] = np.broadcast_to(l2b, (P, H)).copy()

    in_maps = []
    for b in range(B):
        m = dict(common)
        m["xT"] = tile_rows(x[b].T)
        m["x_res"] = (x[b] + bo_p[None, :]).astype(f16)
        in_maps.append(m)
    return in_maps, trivial_ln1, trivial_ln2, x.dtype


def run(inputs, trace=False, **kw):
    in_maps, t1, t2, dt = _prepare(inputs)
    nc = _get_nc(t1, t2)
    last_err = None
    for attempt in range(3):
        try:
            res = run_bass_kernel_spmd(
                nc, in_maps, core_ids=list(range(len(in_maps))),
                trace=trace, **kw,
            )
            break
        except Exception as e:  # transient NRT_EXEC_UNIT_UNRECOVERABLE etc.
            last_err = e
            import time as _time

            _time.sleep(10)
    else:
        raise last_err
    out = np.stack([res.results[c]["out"] for c in range(len(in_maps))])
    return out.astype(dt, copy=False), res


def kernel(**inputs) -> np.ndarray:
    out, _ = run(inputs, trace=False)
    return out
